# revision 24
# baseline (speedup 1.0000x reference)
"""Trainium2 Bass kernel for the EnsembleFeatureLoss OT problem.

Math (per ensemble member e of E=4):
  s = l2norm_rows(gts[e]); t = l2norm_rows(feats[e])      # [4096, 1024]
  sim = s @ t.T                                            # [4096, 4096]
  K = exp(10*sim - 10)
  Sinkhorn converges in exactly 2 iterations for this regime (verified
  against the reference with ~1e5x margin on both sides of the 0.01
  threshold; re-verified on the host from kernel outputs, with a full
  numpy fallback if that check ever fails):
    r1 = u / rowsum(K);  c1 = v / (K.T @ r1)
    r2 = u / (K @ c1);   c2 = v / (K.T @ r2)
  loss_e = sum(outer(r2, c2) * K * sim) = c2 . Z,  Z[n] = sum_m r2 K sim

Distribution: 8 cores = 4 members x 2 row-halves (2048 rows each).
Each core runs two fused passes over its [2048, 4096] block:
  pass A: bf16 matmul -> sim' chunks -> (bf16 spill to DRAM) -> exp with
          fused rowsum accum -> r1 -> P1 += K*r1 (fused STT).
  pair AllReduce of Y1 = colsum(P1) -> c1 (16KB collective).
  pass B: reload sim', exp, fused c1-weighted row-dots -> r2, and the
          P2 / PZ accumulators; Y2/Z colsums via PE ones-matmul.
Host combines per-core [4096] partial vectors (O(N) work only) and does
the 4-scalar ensemble weighting.

Normalization trick: operands stay *unnormalized* bf16; 1/|t| is folded
into the tT operand, and 1/|s| rides the per-partition scale AP of the
ACT exp (K = exp(10*inv_s[m]*sim' - 10)) and the r2 scalar of the Z
accumulator. inv-norms use exp(-0.5*ln(x)) (Ln/Exp are ~2ULP) instead of
the loose-tolerance Sqrt table.
"""

import numpy as np
import ml_dtypes

BF16 = ml_dtypes.bfloat16
F8 = ml_dtypes.float8_e4m3
SIGMA = 16.0                # fp8 operand pre-scale; sim' = SIGMA^2 * sim

E = 4
M = 4096
N = 4096
D = 1024
P = 128
NCORES = 8
MHALF = M // 2              # rows per core
CH = 512                    # psum chunk (one fp32 bank)
RB = 2                      # pass-A row-tiles kept resident for pass B

_CACHE = {}


def build_bass(mhalf=MHALF, n=N, d=D, ncores=NCORES, m_total=None):
    import concourse.bass as bass
    import concourse.mybir as mybir
    import concourse.tile as tile
    from concourse import bacc
    from concourse.bass import ts

    dt = mybir.dt
    f32, bf16 = dt.float32, dt.bfloat16
    Alu = mybir.AluOpType
    Act = mybir.ActivationFunctionType

    if m_total is None:
        m_total = 2 * mhalf
    nt_m = mhalf // P
    nd = d // P
    nch = n // CH
    nh = n // 2
    spill_mis = list(range(mhalf // P - RB))
    mis_b = list(range(mhalf // P - RB, mhalf // P)) + spill_mis
    u32 = float(np.float32(1.0 / m_total))
    v32 = float(np.float32(1.0 / n))
    rg = [[i, i + 1] for i in range(0, ncores, 2)]

    nc = bacc.Bacc("TRN2", target_bir_lowering=False, debug=False,
                   num_devices=ncores)
    f8 = dt.float8e4
    nd2 = d // 256
    sT = nc.declare_dram_parameter("sT", [nd2, P, 2, mhalf], f8, isOutput=False)
    tT = nc.declare_dram_parameter("tT", [nd2, P, 2, n], f8, isOutput=False)
    vecs = nc.declare_dram_parameter("vecs", [2, n], f32, isOutput=True)
    r1o = nc.declare_dram_parameter("r1o", [P, nt_m], f32, isOutput=True)
    r2o = nc.declare_dram_parameter("r2o", [P, nt_m], f32, isOutput=True)

    with tile.TileContext(nc) as tc:
        with (
            tc.tile_pool(name="persist", bufs=1) as pp,
            tc.tile_pool(name="opt", bufs=12) as optp,     # tT blocks / pass-B big tiles
            tc.tile_pool(name="ops", bufs=8) as opsp,     # sT blocks
            tc.tile_pool(name="prol", bufs=2) as prolp,   # squares / invt_bc / pass-A K
            tc.tile_pool(name="stage", bufs=RB) as stagep,  # sim bf16 staging tiles
            tc.tile_pool(name="vec", bufs=1) as vecp,     # [1,N]-ish fp32 vectors
            tc.tile_pool(name="vech", bufs=1) as vechp,   # [1,N] bf16 vectors
            tc.tile_pool(name="kc", bufs=2) as kcp,
            tc.tile_pool(name="sm", bufs=8) as smp,       # tiny per-tile stats
            tc.tile_pool(name="ps", bufs=8, space="PSUM") as psp,
            tc.tile_pool(name="dram", bufs=1, space="DRAM") as dp,
        ):
            # ---- dram scratch ----
            simd = dp.tile([mhalf, n], bf16, name="simd", tag="simd")
            y1_in = dp.tile([1, n], f32, name="y1_in", tag="y1_in")
            y1_out = dp.tile([1, n], f32, name="y1_out", tag="y1_out")
            c1_d = dp.tile([1, n], bf16, name="c1_d", tag="c1_d")

            # ---- persistent sbuf ----
            tTb = [optp.tile([P, 2, n], f8, name=f"tTb{b}", tag="opt")
                   for b in range(nd2)]
            sTb = [opsp.tile([P, 2, mhalf], f8, name=f"sTb{b}", tag="ops")
                   for b in range(nd2)]
            c1_bc = pp.tile([P, n], bf16, name="c1_bc", tag="c1_bc")
            P1 = pp.tile([P, n], bf16, name="P1", tag="P1")
            ones = pp.tile([P, 1], bf16, name="ones", tag="ones")
            scale10 = pp.tile([P, 1], f32, name="scale10", tag="scale10")
            r1buf = pp.tile([P, nt_m], f32, name="r1buf", tag="r1buf")
            r2buf = pp.tile([P, nt_m], f32, name="r2buf", tag="r2buf")
            biasm10 = pp.tile([P, 1], f32, name="biasm10", tag="biasm10")

            nc.vector.memset(biasm10[:], -10.0)
            nc.vector.memset(scale10[:], 10.0 / (16.0 * 16.0))
            nc.vector.memset(ones[:], 1.0)
            nc.vector.memset(P1[:], 0.0)

            # ---- input loads, interleaved t/s per contraction block ----
            from concourse.tile import add_dep_helper
            prev = None
            for b in range(nd2):
                i1 = nc.sync.dma_start(tTb[b][:], tT[b])
                if prev is not None:
                    add_dep_helper(i1.ins, prev.ins, sync=True,
                                   reason="block-ordered input stream")
                i2 = nc.sync.dma_start(sTb[b][:], sT[b])
                prev = i2

            # ---- pass A ----
            resident = {}
            for mi in range(nt_m):
                stage = stagep.tile([P, n], bf16, name="stage", tag="stage")
                K = prolp.tile([P, n], bf16, name="K", tag="prol")
                rs8 = smp.tile([P, nch], f32, name="rs8", tag="sm")
                for ni in range(nch):
                    pm = psp.tile([P, CH], f32, name="pm", tag="ps")
                    for dd in range(nd2):
                        nc.tensor.matmul(
                            pm[:],
                            sTb[dd][:, :, ts(mi, P)],
                            tTb[dd][:, :, ts(ni, CH)],
                            start=(dd == 0), stop=(dd == nd2 - 1),
                            perf_mode=mybir.MatmulPerfMode.DoubleRow)
                    if ni % 2 == 0:
                        nc.scalar.copy(stage[:, ts(ni, CH)], pm[:])
                    else:
                        nc.vector.tensor_copy(stage[:, ts(ni, CH)], pm[:])
                    nc.scalar.activation(K[:, ts(ni, CH)], pm[:], Act.Exp,
                                         bias=biasm10[:],
                                         scale=scale10[:],
                                         accum_out=rs8[:, ni:ni + 1])
                if mi in spill_mis:
                    nc.sync.dma_start(simd[ts(mi, P), :], stage[:])
                else:
                    resident[mi] = stage
                rowsum = smp.tile([P, 1], f32, name="rowsum", tag="sm")
                nc.vector.tensor_reduce(rowsum[:], rs8[:],
                                        mybir.AxisListType.X, Alu.add)
                rinv = smp.tile([P, 1], f32, name="rinv", tag="sm")
                nc.vector.reciprocal(rinv[:], rowsum[:])
                nc.vector.tensor_scalar_mul(r1buf[:, mi:mi + 1], rinv[:], u32)
                nc.vector.scalar_tensor_tensor(
                    out=P1[:], in0=K[:], scalar=r1buf[:, mi:mi + 1],
                    in1=P1[:], op0=Alu.mult, op1=Alu.add)

            # ---- pass-B reloads: sync queue, half tiles into the sT pool
            # (only the lookahead window streams during the collective; the
            # rest wait for the AR so its 16KB isn't starved by bulk DMA)
            reloads = {}
            rl_dmas = []
            for mi in spill_mis:
                h0 = opsp.tile([P, nh], bf16, name=f"rl{mi}a", tag="ops")
                h1 = opsp.tile([P, nh], bf16, name=f"rl{mi}b", tag="ops")
                d0 = nc.sync.dma_start(h0[:], simd[ts(mi, P), 0:nh])
                d1 = nc.sync.dma_start(h1[:], simd[ts(mi, P), nh:n])
                rl_dmas.append((mi, d0, d1))
                reloads[mi] = (h0, h1)

            nc.gpsimd.dma_start(r1o[:, :], r1buf[:])

            # ---- Y1 = colsum(P1); pair AllReduce; c1 ----
            y1sb = vecp.tile([1, n], f32, name="y1sb", tag="vec")
            yb = [psp.tile([P, CH], f32, name=f"yb{i}", tag="ps")
                  for i in range(nch // 2)]
            for i in range(nch // 2):
                nc.vector.memset(yb[i][:], 0.0)
            for c in range(nch):
                b, q = divmod(c, 2)
                nc.tensor.matmul(yb[b][32 * q:32 * q + 1, :], ones[:],
                                 P1[:, ts(c, CH)],
                                 start=False, stop=True,
                                 skip_group_check=True)
            for c in range(nch):
                b, q = divmod(c, 2)
                nc.scalar.copy(y1sb[0:1, ts(c, CH)],
                               yb[b][32 * q:32 * q + 1, :])
            nc.gpsimd.dma_start(y1_in[:], y1sb[0:1, :])
            nc.gpsimd.collective_compute(
                "AllReduce", Alu.add, replica_groups=rg,
                ins=[y1_in.opt()], outs=[y1_out.opt()])
            y1l = vecp.tile([nch, CH], f32, name="y1l", tag="y1l")
            y1l_dma = nc.gpsimd.dma_start(
                y1l[:], y1_out[0:1, :].rearrange("a (c f) -> (a c) f", c=nch))
            ngate = 4
            for mi, d0, d1 in rl_dmas[ngate:]:
                add_dep_helper(d0.ins, y1l_dma.ins, sync=True,
                               reason="yield DMA to the collective")
            nc.vector.reciprocal(y1l[:], y1l[:])
            c1s = vecp.tile([nch, CH], bf16, name="c1s", tag="c1s")
            nc.vector.tensor_scalar_mul(c1s[:], y1l[:], v32)
            nc.gpsimd.dma_start(
                c1_d[0:1, :].rearrange("a (c f) -> (a c) f", c=nch), c1s[:])
            nc.scalar.dma_start(c1_bc[:], c1_d[0:1, :].to_broadcast((P, n)))

            # ---- pass B ----
            # Y2/Z accumulate on the (otherwise idle) PE: per column chunk
            # one psum bank holds Y2 at partition 0 and Z at partition 32.
            # Banks are zeroed by DVE and all matmuls run start=False, so
            # first-touch overwrite/accumulate is order- and state-proof.
            pyz = [psp.tile([P, CH], f32, name=f"pyz{c}", tag="ps")
                   for c in range(nch)]
            for c in range(nch):
                nc.vector.memset(pyz[c][:], 0.0)
            last_mi = mis_b[-1]
            QLA = 6                      # tiles of exp/kx lookahead

            def _halves_of(mi):
                if mi in resident:
                    st = resident[mi]
                    return (st[:, 0:nh], st[:, nh:n])
                rl = reloads[mi]
                return (rl[0][:], rl[1][:])

            K2s, kxs = {}, {}

            def _emit_indep(mi):
                halves = _halves_of(mi)
                K2 = optp.tile([P, n], bf16, name="K2", tag="opt")
                for h in range(2):
                    nc.scalar.activation(K2[:, h * nh:(h + 1) * nh], halves[h],
                                         Act.Exp, bias=biasm10[:],
                                         scale=scale10[:])
                kx = optp.tile([P, n], bf16, name="kx", tag="opt")
                for h in range(2):
                    nc.vector.tensor_mul(kx[:, h * nh:(h + 1) * nh],
                                         K2[:, h * nh:(h + 1) * nh], halves[h])
                K2s[mi], kxs[mi] = K2, kx

            for mi in mis_b[:QLA]:
                _emit_indep(mi)
            for j, mi in enumerate(mis_b):
                K2, kx = K2s.pop(mi), kxs.pop(mi)
                kcL = kcp.tile([P, nh], bf16, name="kcL", tag="kc")
                kcH = kcp.tile([P, nh], bf16, name="kcH", tag="kc")
                nc.vector.tensor_mul(kcL[:], K2[:, 0:nh], c1_bc[:, 0:nh])
                nc.gpsimd.tensor_mul(kcH[:], K2[:, nh:n], c1_bc[:, nh:n])
                rowdot = smp.tile([P, 1], f32, name="rowdot", tag="sm")
                rda = smp.tile([P, 1], f32, name="rda", tag="sm")
                rdb = smp.tile([P, 1], f32, name="rdb", tag="sm")
                nc.vector.tensor_reduce(rda[:], kcL[:],
                                        mybir.AxisListType.X, Alu.add)
                nc.scalar.activation(kcH[:], kcH[:],
                                     Act.Copy, accum_out=rdb[:])
                nc.vector.tensor_add(rowdot[:], rda[:], rdb[:])
                rdinv = smp.tile([P, 1], f32, name="rdinv", tag="sm")
                nc.vector.reciprocal(rdinv[:], rowdot[:])
                nc.vector.tensor_scalar_mul(r2buf[:, mi:mi + 1], rdinv[:], u32)
                r2h = smp.tile([P, 1], bf16, name="r2h", tag="smh")
                nc.vector.tensor_copy(r2h[:], r2buf[:, mi:mi + 1])
                for c in range(nch):
                    nc.tensor.matmul(pyz[c][0:1, :],
                                     r2h[:], K2[:, ts(c, CH)],
                                     start=False, stop=(mi == last_mi),
                                     skip_group_check=True)
                    nc.tensor.matmul(pyz[c][32:33, :],
                                     r2h[:], kx[:, ts(c, CH)],
                                     start=False, stop=(mi == last_mi),
                                     skip_group_check=True)
                if j + QLA < len(mis_b):
                    _emit_indep(mis_b[j + QLA])

            # ---- outputs ----
            y2sb = vecp.tile([1, n], f32, name="y2sb", tag="vec")
            zsb = vecp.tile([1, n], f32, name="zsb", tag="vec")
            for c in range(nch):
                nc.scalar.copy(y2sb[0:1, ts(c, CH)], pyz[c][0:1, :])
                nc.vector.tensor_copy(zsb[0:1, ts(c, CH)], pyz[c][32:33, :])
            nc.sync.dma_start(vecs[0:1, :], y2sb[0:1, :])
            nc.sync.dma_start(vecs[1:2, :], zsb[0:1, :])
            nc.gpsimd.dma_start(r2o[:, :], r2buf[:])

    return nc


def _normalize_rows(x):
    x = np.asarray(x, np.float32)
    nrm = np.sqrt((x * x).sum(axis=1, keepdims=True))
    return x / np.maximum(nrm, 1e-12)


def _pair_pack(xT):
    """[D, C] -> [D//256, 128, 2, C] DoubleRow operand layout (fp8)."""
    Dd, C = xT.shape
    return np.ascontiguousarray(
        xT.reshape(Dd // 256, 2, P, C).transpose(0, 2, 1, 3))


def _make_in_maps(gts, feats):
    in_maps = []
    sn = [_normalize_rows(gts[e]) for e in range(E)]
    tn8 = [_pair_pack((SIGMA * _normalize_rows(feats[e]).T).astype(F8))
           for e in range(E)]
    for core in range(NCORES):
        e, h = divmod(core, 2)
        s_half = sn[e][h * MHALF:(h + 1) * MHALF]          # [2048, 1024]
        in_maps.append({
            "sT": _pair_pack((SIGMA * s_half.T).astype(F8)),
            "tT": tn8[e],
        })
    return in_maps


def _ensemble(losses, prev_losses):
    l = np.asarray(losses, np.float64)
    ratio = l / (np.asarray(prev_losses, np.float64) + 1e-8)
    w = np.exp(ratio / 1.0)
    w = w / np.sum(w) * l.shape[0]
    return np.float32(np.sum(w * l))


def _numpy_reference(gts, feats, prev_losses):
    """Faithful float32 fallback, used only if the on-device convergence
    check is violated (never observed for this problem's regime)."""
    losses = []
    for e in range(gts.shape[0]):
        s = gts[e] / np.maximum(
            np.linalg.norm(gts[e], axis=1, keepdims=True), 1e-12)
        t = feats[e] / np.maximum(
            np.linalg.norm(feats[e], axis=1, keepdims=True), 1e-12)
        sim = (s @ t.T).astype(np.float32)
        K = np.exp(-(1.0 - sim) / 0.1)
        m, n = sim.shape
        u = np.full(m, 1.0 / m, np.float32)
        v = np.full(n, 1.0 / n, np.float32)
        r = np.ones(m, np.float32)
        c = np.ones(n, np.float32)
        err = np.inf
        for _ in range(100):
            if err < 0.01:
                break
            r_new = u / (K @ c)
            c = v / (K.T @ r_new)
            err = float(np.mean(np.abs(r_new - r)))
            r = r_new
        losses.append(np.sum(np.outer(r, c) * K * sim))
    return _ensemble(losses, prev_losses)


def _run(gts, feats, trace=False):
    from concourse.bass_utils import run_bass_kernel_spmd
    if "nc" not in _CACHE:
        nc = build_bass()
        nc.finalize()
        _CACHE["nc"] = nc
    in_maps = _make_in_maps(gts, feats)
    return run_bass_kernel_spmd(_CACHE["nc"], in_maps,
                                list(range(NCORES)), trace=trace)


def _combine(results, gts, feats, prev_losses):
    losses = []
    ok = True
    for e in range(E):
        a, b = results[2 * e], results[2 * e + 1]
        Y2 = a["vecs"][0].astype(np.float64) + b["vecs"][0].astype(np.float64)
        Z = a["vecs"][1].astype(np.float64) + b["vecs"][1].astype(np.float64)
        c2 = (1.0 / N) / Y2
        losses.append(np.sum(c2 * Z) / (SIGMA * SIGMA))
        r1 = np.concatenate([a["r1o"].T.reshape(-1), b["r1o"].T.reshape(-1)])
        r2 = np.concatenate([a["r2o"].T.reshape(-1), b["r2o"].T.reshape(-1)])
        err1 = np.mean(np.abs(r1 - 1.0))
        err2 = np.mean(np.abs(r2 - r1))
        if not (err1 >= 0.01 and err2 < 0.01):
            ok = False
    if not ok:
        return _numpy_reference(gts, feats, prev_losses)
    return _ensemble(losses, prev_losses)


def kernel(gts, feats, prev_losses):
    gts = np.asarray(gts, np.float32)
    feats = np.asarray(feats, np.float32)
    prev_losses = np.asarray(prev_losses, np.float32)
    res = _run(gts, feats)
    return _combine(res.results, gts, feats, prev_losses)



# revision 27
# speedup vs baseline: 1.0793x; 1.0793x over previous
"""Trainium2 Bass kernel for the EnsembleFeatureLoss OT problem.

Math (per ensemble member e of E=4):
  s = l2norm_rows(gts[e]); t = l2norm_rows(feats[e])      # [4096, 1024]
  sim = s @ t.T                                            # [4096, 4096]
  K = exp(10*sim - 10)
  Sinkhorn converges in exactly 2 iterations for this regime (verified
  against the reference with ~1e5x margin on both sides of the 0.01
  threshold; re-verified on the host from kernel outputs, with a full
  numpy fallback if that check ever fails):
    r1 = u / rowsum(K);  c1 = v / (K.T @ r1)
    r2 = u / (K @ c1);   c2 = v / (K.T @ r2)
  loss_e = sum(outer(r2, c2) * K * sim) = c2 . Z,  Z[n] = sum_m r2 K sim

Distribution: 8 cores = 4 members x 2 row-halves (2048 rows each).
Each core runs two fused passes over its [2048, 4096] block:
  pass A: bf16 matmul -> sim' chunks -> (bf16 spill to DRAM) -> exp with
          fused rowsum accum -> r1 -> P1 += K*r1 (fused STT).
  pair AllReduce of Y1 = colsum(P1) -> c1 (16KB collective).
  pass B: reload sim', exp, fused c1-weighted row-dots -> r2, and the
          P2 / PZ accumulators; Y2/Z colsums via PE ones-matmul.
Host combines per-core [4096] partial vectors (O(N) work only) and does
the 4-scalar ensemble weighting.

Normalization trick: operands stay *unnormalized* bf16; 1/|t| is folded
into the tT operand, and 1/|s| rides the per-partition scale AP of the
ACT exp (K = exp(10*inv_s[m]*sim' - 10)) and the r2 scalar of the Z
accumulator. inv-norms use exp(-0.5*ln(x)) (Ln/Exp are ~2ULP) instead of
the loose-tolerance Sqrt table.
"""

import numpy as np
import ml_dtypes

BF16 = ml_dtypes.bfloat16
F8 = ml_dtypes.float8_e4m3
SIGMA = 16.0                # fp8 operand pre-scale; sim' = SIGMA^2 * sim

E = 4
M = 4096
N = 4096
D = 1024
P = 128
NCORES = 8
MHALF = M // 2              # rows per core
CH = 512                    # psum chunk (one fp32 bank)
RB = 2                      # pass-A row-tiles kept resident for pass B

_CACHE = {}


def build_bass(mhalf=MHALF, n=N, d=D, ncores=NCORES, m_total=None):
    import concourse.bass as bass
    import concourse.mybir as mybir
    import concourse.tile as tile
    from concourse import bacc
    from concourse.bass import ts

    dt = mybir.dt
    f32, bf16 = dt.float32, dt.bfloat16
    Alu = mybir.AluOpType
    Act = mybir.ActivationFunctionType

    if m_total is None:
        m_total = 2 * mhalf
    nt_m = mhalf // P
    nd = d // P
    nch = n // CH
    nh = n // 2
    spill_mis = list(range(mhalf // P - RB))
    mis_b = list(range(mhalf // P - RB, mhalf // P)) + spill_mis
    u32 = float(np.float32(1.0 / m_total))
    v32 = float(np.float32(1.0 / n))
    rg = [[i, i + 1] for i in range(0, ncores, 2)]

    nc = bacc.Bacc("TRN2", target_bir_lowering=False, debug=False,
                   num_devices=ncores)
    f8 = dt.float8e4
    nd2 = d // 256
    sT = nc.declare_dram_parameter("sT", [nd2, P, 2, mhalf], f8, isOutput=False)
    tT = nc.declare_dram_parameter("tT", [nd2, P, 2, n], f8, isOutput=False)
    vecs = nc.declare_dram_parameter("vecs", [2, n], f32, isOutput=True)
    r1o = nc.declare_dram_parameter("r1o", [P, nt_m], f32, isOutput=True)
    r2o = nc.declare_dram_parameter("r2o", [P, nt_m], f32, isOutput=True)

    with tile.TileContext(nc) as tc:
        with (
            tc.tile_pool(name="persist", bufs=1) as pp,
            tc.tile_pool(name="opt", bufs=12) as optp,     # tT blocks / pass-B big tiles
            tc.tile_pool(name="ops", bufs=8) as opsp,     # sT blocks
            tc.tile_pool(name="prol", bufs=2) as prolp,   # squares / invt_bc / pass-A K
            tc.tile_pool(name="stage", bufs=RB) as stagep,  # sim bf16 staging tiles
            tc.tile_pool(name="vec", bufs=1) as vecp,     # [1,N]-ish fp32 vectors
            tc.tile_pool(name="vech", bufs=1) as vechp,   # [1,N] bf16 vectors
            tc.tile_pool(name="kc", bufs=2) as kcp,
            tc.tile_pool(name="sm", bufs=8) as smp,       # tiny per-tile stats
            tc.tile_pool(name="ps", bufs=8, space="PSUM") as psp,
            tc.tile_pool(name="dram", bufs=1, space="DRAM") as dp,
        ):
            # ---- dram scratch ----
            simd = dp.tile([mhalf, n], bf16, name="simd", tag="simd")
            y1_in = dp.tile([1, n], f32, name="y1_in", tag="y1_in")
            y1_out = dp.tile([1, n], f32, name="y1_out", tag="y1_out")
            c1_d = dp.tile([1, n], bf16, name="c1_d", tag="c1_d")

            # ---- persistent sbuf ----
            tTb = [optp.tile([P, 2, n], f8, name=f"tTb{b}", tag="opt")
                   for b in range(nd2)]
            sTb = [opsp.tile([P, 2, mhalf], f8, name=f"sTb{b}", tag="ops")
                   for b in range(nd2)]
            c1_bc = pp.tile([P, n], bf16, name="c1_bc", tag="c1_bc")
            P1 = pp.tile([P, n], bf16, name="P1", tag="P1")
            ones = pp.tile([P, 1], bf16, name="ones", tag="ones")
            scale10 = pp.tile([P, 1], f32, name="scale10", tag="scale10")
            r1buf = pp.tile([P, nt_m], f32, name="r1buf", tag="r1buf")
            r2buf = pp.tile([P, nt_m], f32, name="r2buf", tag="r2buf")
            biasm10 = pp.tile([P, 1], f32, name="biasm10", tag="biasm10")

            nc.vector.memset(biasm10[:], -10.0)
            nc.vector.memset(scale10[:], 10.0 / (16.0 * 16.0))
            nc.vector.memset(ones[:], 1.0)
            nc.vector.memset(P1[:], 0.0)

            # ---- input loads, interleaved t/s per contraction block ----
            from concourse.tile import add_dep_helper
            prev = None
            for b in range(nd2):
                i1 = nc.sync.dma_start(tTb[b][:], tT[b])
                if prev is not None:
                    add_dep_helper(i1.ins, prev.ins, sync=True,
                                   reason="block-ordered input stream")
                i2 = nc.sync.dma_start(sTb[b][:], sT[b])
                prev = i2

            # ---- pass A ----
            resident = {}
            for mi in range(nt_m):
                stage = stagep.tile([P, n], bf16, name="stage", tag="stage")
                K = prolp.tile([P, n], bf16, name="K", tag="prol")
                rs8 = smp.tile([P, nch], f32, name="rs8", tag="sm")
                for ni in range(nch):
                    pm = psp.tile([P, CH], f32, name="pm", tag="ps")
                    for dd in range(nd2):
                        nc.tensor.matmul(
                            pm[:],
                            sTb[dd][:, :, ts(mi, P)],
                            tTb[dd][:, :, ts(ni, CH)],
                            start=(dd == 0), stop=(dd == nd2 - 1),
                            perf_mode=mybir.MatmulPerfMode.DoubleRow)
                    if ni % 2 == 0:
                        nc.scalar.copy(stage[:, ts(ni, CH)], pm[:])
                    else:
                        nc.vector.tensor_copy(stage[:, ts(ni, CH)], pm[:])
                    nc.scalar.activation(K[:, ts(ni, CH)], pm[:], Act.Exp,
                                         bias=biasm10[:],
                                         scale=scale10[:],
                                         accum_out=rs8[:, ni:ni + 1])
                if mi in spill_mis:
                    nc.sync.dma_start(simd[ts(mi, P), :], stage[:])
                else:
                    resident[mi] = stage
                rowsum = smp.tile([P, 1], f32, name="rowsum", tag="sm")
                nc.vector.tensor_reduce(rowsum[:], rs8[:],
                                        mybir.AxisListType.X, Alu.add)
                rinv = smp.tile([P, 1], f32, name="rinv", tag="sm")
                nc.vector.reciprocal(rinv[:], rowsum[:])
                nc.vector.tensor_scalar_mul(r1buf[:, mi:mi + 1], rinv[:], u32)
                nc.vector.scalar_tensor_tensor(
                    out=P1[:], in0=K[:], scalar=r1buf[:, mi:mi + 1],
                    in1=P1[:], op0=Alu.mult, op1=Alu.add)

            # ---- pass-B reloads: sync queue, half tiles into the sT pool
            # (only the lookahead window streams during the collective; the
            # rest wait for the AR so its 16KB isn't starved by bulk DMA)
            reloads = {}
            rl_dmas = []
            for mi in spill_mis:
                h0 = opsp.tile([P, nh], bf16, name=f"rl{mi}a", tag="ops")
                h1 = opsp.tile([P, nh], bf16, name=f"rl{mi}b", tag="ops")
                d0 = nc.sync.dma_start(h0[:], simd[ts(mi, P), 0:nh])
                d1 = nc.sync.dma_start(h1[:], simd[ts(mi, P), nh:n])
                rl_dmas.append((mi, d0, d1))
                reloads[mi] = (h0, h1)

            nc.gpsimd.dma_start(r1o[:, :], r1buf[:])

            # ---- Y1 = colsum(P1); pair AllReduce; c1 ----
            y1sb = vecp.tile([1, n], f32, name="y1sb", tag="vec")
            yb = [psp.tile([P, CH], f32, name=f"yb{i}", tag="ps")
                  for i in range(nch // 2)]
            for i in range(nch // 2):
                nc.vector.memset(yb[i][:], 0.0)
            for c in range(nch):
                b, q = divmod(c, 2)
                nc.tensor.matmul(yb[b][32 * q:32 * q + 1, :], ones[:],
                                 P1[:, ts(c, CH)],
                                 start=False, stop=True,
                                 skip_group_check=True)
            for c in range(nch):
                b, q = divmod(c, 2)
                nc.scalar.copy(y1sb[0:1, ts(c, CH)],
                               yb[b][32 * q:32 * q + 1, :])
            nc.gpsimd.dma_start(y1_in[:], y1sb[0:1, :])
            nc.gpsimd.collective_compute(
                "AllReduce", Alu.add, replica_groups=rg,
                ins=[y1_in.opt()], outs=[y1_out.opt()])
            y1l = vecp.tile([nch, CH], f32, name="y1l", tag="y1l")
            y1l_dma = nc.gpsimd.dma_start(
                y1l[:], y1_out[0:1, :].rearrange("a (c f) -> (a c) f", c=nch))
            ngate = 4
            for mi, d0, d1 in rl_dmas[ngate:]:
                add_dep_helper(d0.ins, y1l_dma.ins, sync=True,
                               reason="yield DMA to the collective")
            nc.vector.reciprocal(y1l[:], y1l[:])
            c1s = vecp.tile([nch, CH], bf16, name="c1s", tag="c1s")
            nc.vector.tensor_scalar_mul(c1s[:], y1l[:], v32)
            nc.gpsimd.dma_start(
                c1_d[0:1, :].rearrange("a (c f) -> (a c) f", c=nch), c1s[:])
            nc.scalar.dma_start(c1_bc[:], c1_d[0:1, :].to_broadcast((P, n)))

            # ---- pass B ----
            # Y2/Z accumulate on the (otherwise idle) PE: per column chunk
            # one psum bank holds Y2 at partition 0 and Z at partition 32.
            # Banks are zeroed by DVE and all matmuls run start=False, so
            # first-touch overwrite/accumulate is order- and state-proof.
            pyz = [psp.tile([P, CH], f32, name=f"pyz{c}", tag="ps")
                   for c in range(nch)]
            for c in range(nch):
                nc.vector.memset(pyz[c][:], 0.0)
            last_mi = mis_b[-1]
            QLA = 6                      # tiles of exp/kx lookahead

            def _halves_of(mi):
                if mi in resident:
                    st = resident[mi]
                    return (st[:, 0:nh], st[:, nh:n])
                rl = reloads[mi]
                return (rl[0][:], rl[1][:])

            K2s, kxs = {}, {}

            def _emit_indep(mi):
                halves = _halves_of(mi)
                K2 = optp.tile([P, n], bf16, name="K2", tag="opt")
                for h in range(2):
                    nc.scalar.activation(K2[:, h * nh:(h + 1) * nh], halves[h],
                                         Act.Exp, bias=biasm10[:],
                                         scale=scale10[:])
                kx = optp.tile([P, n], bf16, name="kx", tag="opt")
                for h in range(2):
                    nc.vector.tensor_mul(kx[:, h * nh:(h + 1) * nh],
                                         K2[:, h * nh:(h + 1) * nh], halves[h])
                K2s[mi], kxs[mi] = K2, kx

            for mi in mis_b[:QLA]:
                _emit_indep(mi)
            for j, mi in enumerate(mis_b):
                K2, kx = K2s.pop(mi), kxs.pop(mi)
                kcL = kcp.tile([P, nh], bf16, name="kcL", tag="kc")
                kcH = kcp.tile([P, nh], bf16, name="kcH", tag="kc")
                nc.vector.tensor_mul(kcL[:], K2[:, 0:nh], c1_bc[:, 0:nh])
                nc.vector.tensor_mul(kcH[:], K2[:, nh:n], c1_bc[:, nh:n])
                rowdot = smp.tile([P, 1], f32, name="rowdot", tag="sm")
                rda = smp.tile([P, 1], f32, name="rda", tag="sm")
                rdb = smp.tile([P, 1], f32, name="rdb", tag="sm")
                nc.vector.tensor_reduce(rda[:], kcL[:],
                                        mybir.AxisListType.X, Alu.add)
                nc.scalar.activation(kcH[:], kcH[:],
                                     Act.Copy, accum_out=rdb[:])
                nc.vector.tensor_add(rowdot[:], rda[:], rdb[:])
                rdinv = smp.tile([P, 1], f32, name="rdinv", tag="sm")
                nc.vector.reciprocal(rdinv[:], rowdot[:])
                nc.vector.tensor_scalar_mul(r2buf[:, mi:mi + 1], rdinv[:], u32)
                r2h = smp.tile([P, 1], bf16, name="r2h", tag="smh")
                nc.scalar.copy(r2h[:], r2buf[:, mi:mi + 1])
                for c in range(nch):
                    nc.tensor.matmul(pyz[c][0:1, :],
                                     r2h[:], K2[:, ts(c, CH)],
                                     start=False, stop=(mi == last_mi),
                                     skip_group_check=True)
                    nc.tensor.matmul(pyz[c][32:33, :],
                                     r2h[:], kx[:, ts(c, CH)],
                                     start=False, stop=(mi == last_mi),
                                     skip_group_check=True)
                if j + QLA < len(mis_b):
                    _emit_indep(mis_b[j + QLA])

            # ---- outputs ----
            y2sb = vecp.tile([1, n], f32, name="y2sb", tag="vec")
            zsb = vecp.tile([1, n], f32, name="zsb", tag="vec")
            for c in range(nch):
                nc.scalar.copy(y2sb[0:1, ts(c, CH)], pyz[c][0:1, :])
                nc.vector.tensor_copy(zsb[0:1, ts(c, CH)], pyz[c][32:33, :])
            nc.sync.dma_start(vecs[0:1, :], y2sb[0:1, :])
            nc.sync.dma_start(vecs[1:2, :], zsb[0:1, :])
            nc.gpsimd.dma_start(r2o[:, :], r2buf[:])

    return nc


def _normalize_rows(x):
    x = np.asarray(x, np.float32)
    nrm = np.sqrt((x * x).sum(axis=1, keepdims=True))
    return x / np.maximum(nrm, 1e-12)


def _pair_pack(xT):
    """[D, C] -> [D//256, 128, 2, C] DoubleRow operand layout (fp8)."""
    Dd, C = xT.shape
    return np.ascontiguousarray(
        xT.reshape(Dd // 256, 2, P, C).transpose(0, 2, 1, 3))


def _make_in_maps(gts, feats):
    in_maps = []
    sn = [_normalize_rows(gts[e]) for e in range(E)]
    tn8 = [_pair_pack((SIGMA * _normalize_rows(feats[e]).T).astype(F8))
           for e in range(E)]
    for core in range(NCORES):
        e, h = divmod(core, 2)
        s_half = sn[e][h * MHALF:(h + 1) * MHALF]          # [2048, 1024]
        in_maps.append({
            "sT": _pair_pack((SIGMA * s_half.T).astype(F8)),
            "tT": tn8[e],
        })
    return in_maps


def _ensemble(losses, prev_losses):
    l = np.asarray(losses, np.float64)
    ratio = l / (np.asarray(prev_losses, np.float64) + 1e-8)
    w = np.exp(ratio / 1.0)
    w = w / np.sum(w) * l.shape[0]
    return np.float32(np.sum(w * l))


def _numpy_reference(gts, feats, prev_losses):
    """Faithful float32 fallback, used only if the on-device convergence
    check is violated (never observed for this problem's regime)."""
    losses = []
    for e in range(gts.shape[0]):
        s = gts[e] / np.maximum(
            np.linalg.norm(gts[e], axis=1, keepdims=True), 1e-12)
        t = feats[e] / np.maximum(
            np.linalg.norm(feats[e], axis=1, keepdims=True), 1e-12)
        sim = (s @ t.T).astype(np.float32)
        K = np.exp(-(1.0 - sim) / 0.1)
        m, n = sim.shape
        u = np.full(m, 1.0 / m, np.float32)
        v = np.full(n, 1.0 / n, np.float32)
        r = np.ones(m, np.float32)
        c = np.ones(n, np.float32)
        err = np.inf
        for _ in range(100):
            if err < 0.01:
                break
            r_new = u / (K @ c)
            c = v / (K.T @ r_new)
            err = float(np.mean(np.abs(r_new - r)))
            r = r_new
        losses.append(np.sum(np.outer(r, c) * K * sim))
    return _ensemble(losses, prev_losses)


def _run(gts, feats, trace=False):
    from concourse.bass_utils import run_bass_kernel_spmd
    if "nc" not in _CACHE:
        nc = build_bass()
        nc.finalize()
        _CACHE["nc"] = nc
    in_maps = _make_in_maps(gts, feats)
    return run_bass_kernel_spmd(_CACHE["nc"], in_maps,
                                list(range(NCORES)), trace=trace)


def _combine(results, gts, feats, prev_losses):
    losses = []
    ok = True
    for e in range(E):
        a, b = results[2 * e], results[2 * e + 1]
        Y2 = a["vecs"][0].astype(np.float64) + b["vecs"][0].astype(np.float64)
        Z = a["vecs"][1].astype(np.float64) + b["vecs"][1].astype(np.float64)
        c2 = (1.0 / N) / Y2
        losses.append(np.sum(c2 * Z) / (SIGMA * SIGMA))
        r1 = np.concatenate([a["r1o"].T.reshape(-1), b["r1o"].T.reshape(-1)])
        r2 = np.concatenate([a["r2o"].T.reshape(-1), b["r2o"].T.reshape(-1)])
        err1 = np.mean(np.abs(r1 - 1.0))
        err2 = np.mean(np.abs(r2 - r1))
        if not (err1 >= 0.01 and err2 < 0.01):
            ok = False
    if not ok:
        return _numpy_reference(gts, feats, prev_losses)
    return _ensemble(losses, prev_losses)


def kernel(gts, feats, prev_losses):
    gts = np.asarray(gts, np.float32)
    feats = np.asarray(feats, np.float32)
    prev_losses = np.asarray(prev_losses, np.float32)
    res = _run(gts, feats)
    return _combine(res.results, gts, feats, prev_losses)

